# revision 8
# baseline (speedup 1.0000x reference)
"""Bahdanau attention on 8 Trainium2 NeuronCores (Bass/Tile).

Data-parallel over batch: B=32 -> 4 batches per core; attn/v weights
replicated. Per core, per batch b:
  enc_proj^T[d, s] = W_enc^T @ enc[b]^T      (bf16 TensorE GEMM)
  energy = tanh(enc_proj^T + r_b)            (ScalarE, r_b = W_dec^T dh_b + b_attn)
  scores[s] = v^T @ energy                   (TensorE, M=1)
  E = exp(scores + (mask-1)*1e9), Z = sum E  (ScalarE exp + accum)
  attn = E / Z                               (VectorE)
  context = attn @ enc[b]                    (TensorE, M=1)

enc^T comes from an f32->bf16 cast DMA (SWDGE) into a DRAM bounce buffer
followed by hardware xbar transpose DMA (2-byte dtype only) back to SBUF.
"""

import os
import sys

import numpy as np

import concourse.bass as bass
import concourse.tile as tile
from concourse import mybir

F32 = mybir.dt.float32
BF16 = mybir.dt.bfloat16
I32 = mybir.dt.int32

N_CORES = 8
B, S, E, D = 32, 2048, 1024, 1024
B_LOC = B // N_CORES
P = 128


def _ensure_ntff_hook():
    """Register the axon NTFF profiling hook if the image lacks antenv.axon_hooks.

    Needed only when profiling (trace=True / BASS_TRACE=1); harmless otherwise.
    """
    try:
        import antenv.axon_hooks  # noqa: F401
        return
    except ImportError:
        pass
    try:
        import types

        import antenv
        from trn_agent_boot.trn_boot import _ntff_profile_via_ctypes

        mod = types.ModuleType("antenv.axon_hooks")
        _state = {"hook": None}

        def set_axon_ntff_profile_hook(hook):
            _state["hook"] = hook

        def get_axon_ntff_profile_hook():
            return _state["hook"]

        mod.set_axon_ntff_profile_hook = set_axon_ntff_profile_hook
        mod.get_axon_ntff_profile_hook = get_axon_ntff_profile_hook
        sys.modules["antenv.axon_hooks"] = mod
        antenv.axon_hooks = mod
        so_path = "/opt/axon/libaxon_pjrt.so"
        if os.path.exists(so_path):
            hook = _ntff_profile_via_ctypes(so_path)
            if hook is not None:
                set_axon_ntff_profile_hook(hook)
    except Exception:
        pass


def _split_multi_waits(nc):
    """Rewrite every instruction with >1 sem wait: keep one wait, hoist the
    rest onto same-engine NOPs inserted immediately before it.

    The pinned walrus codegen supports only ONE sync wait command per
    instruction ('Too many sync wait commands' otherwise); Tile's scheduler
    freely attaches several.
    """
    import bass_rust

    counter = 0
    for bassbb in nc.bb_map.values():
        bb = bassbb.bb
        out = []
        changed = False
        for inst in bb.instructions:
            si = inst.sync_info
            waits = list(si.on_wait) if si is not None else []
            if len(waits) > 1:
                changed = True
                for w in waits[:-1]:
                    counter += 1
                    n = mybir.InstNoOp(name=f"antws-{counter}")
                    n.engine = inst.engine
                    n.sync_info = bass_rust.SyncInfo(on_wait=[w], on_update=[])
                    nc.register_instruction(n, overwrite=True)
                    out.append(n)
                inst.sync_info = bass_rust.SyncInfo(
                    on_wait=[waits[-1]], on_update=list(si.on_update)
                )
            out.append(inst)
        if changed:
            bb.instructions = out


def _patch_tile_tail_drain():
    """Run _split_multi_waits after Tile's tail drain+barrier emission."""
    if getattr(tile.TileContext, "_ant_drain_patched", False):
        return
    orig = tile.TileContext._drain_and_barrier

    def _drain_and_barrier(self, tick_clock, wait_clock):
        orig(self, tick_clock, wait_clock)
        _split_multi_waits(self.nc)

    tile.TileContext._drain_and_barrier = _drain_and_barrier
    tile.TileContext._ant_drain_patched = True


def build_nc(b_loc=B_LOC, s=S, e=E, d=D):
    """Build the per-core Bass program (same SPMD program on all cores)."""
    _patch_tile_tail_drain()
    assert e % P == 0 and d % P == 0 and s % P == 0
    ko_e = e // P  # contraction tiles over enc dim
    ko_d = d // P  # tiles over proj dim d (and dec-hidden contraction)
    nch = max(1, s // 512)  # 512-row score chunks per batch
    ncs = s // nch
    st_n = s // P  # s-tiles per batch (context contraction)
    ech = max(1, e // 512)  # context output chunks
    ecs = e // ech

    nc = bass.Bass("TRN2")
    dh_p = nc.declare_dram_parameter("dh", [b_loc, d], F32, isOutput=False)
    enc_p = nc.declare_dram_parameter("enc", [b_loc, s, e], F32, isOutput=False)
    mask_p = nc.declare_dram_parameter("mask", [b_loc, s], I32, isOutput=False)
    wat_p = nc.declare_dram_parameter("w_attn", [d + e, d], F32, isOutput=False)
    bat_p = nc.declare_dram_parameter("b_attn", [1, d], F32, isOutput=False)
    vw_p = nc.declare_dram_parameter("v_w", [1, d], F32, isOutput=False)
    ctx_p = nc.declare_dram_parameter("ctx_out", [b_loc, e], F32, isOutput=True)
    attn_p = nc.declare_dram_parameter("attn_out", [b_loc, s], F32, isOutput=True)

    Tanh = mybir.ActivationFunctionType.Tanh
    Exp = mybir.ActivationFunctionType.Exp

    from contextlib import ExitStack

    with tile.TileContext(nc) as tc, ExitStack() as ctx:
        dram = ctx.enter_context(tc.tile_pool(name="dram", bufs=1, space="DRAM"))
        singles = ctx.enter_context(tc.tile_pool(name="singles", bufs=1))

        w_enc_sb = singles.tile([P, ko_e, d], BF16)
        nc.gpsimd.dma_start(
            out=w_enc_sb,
            in_=wat_p[d : d + e, :].rearrange("(ko p) n -> p ko n", p=P),
        )
        r_sb = singles.tile([P, ko_d, b_loc], F32)
        vT_sb = singles.tile([P, ko_d, 1], BF16)

        # ---- prep: r = W_dec^T dh^T + b_attn (as [d-part, b]), v^T tiles ----
        with (
            tc.tile_pool(name="prep", bufs=1) as prep,
            tc.tile_pool(name="prep_ps", bufs=2, space="PSUM") as prep_ps,
        ):
            w_dec_sb = prep.tile([P, ko_d, d], BF16)
            nc.gpsimd.dma_start(
                out=w_dec_sb, in_=wat_p[0:d, :].rearrange("(ko p) n -> p ko n", p=P)
            )
            dh_sb = prep.tile([b_loc, d], F32)
            nc.sync.dma_start(out=dh_sb, in_=dh_p[:, :])
            eye_i = prep.tile([b_loc, b_loc], I32)
            nc.gpsimd.iota(eye_i, pattern=[[-1, b_loc]], base=0, channel_multiplier=1)
            eye_sb = prep.tile([b_loc, b_loc], F32)
            nc.vector.tensor_scalar(
                out=eye_sb, in0=eye_i, scalar1=0, scalar2=None,
                op0=mybir.AluOpType.is_equal,
            )
            one_sb = prep.tile([1, 1], F32)
            nc.vector.memset(one_sb, 1.0)
            ones_b = prep.tile([1, b_loc], BF16)
            nc.vector.memset(ones_b, 1.0)
            b_row = prep.tile([1, d], BF16)
            nc.gpsimd.dma_start(out=b_row, in_=bat_p[:, :])
            v_row = prep.tile([1, d], F32)
            nc.sync.dma_start(out=v_row, in_=vw_p[:, :])

            dhT_sb = prep.tile([P, ko_d, b_loc], BF16)
            for ko in range(ko_d):
                ps_t = prep_ps.tile([P, b_loc], F32, tag="ps_t")
                nc.tensor.matmul(
                    ps_t, lhsT=dh_sb[:, ko * P : (ko + 1) * P], rhs=eye_sb,
                    start=True, stop=True,
                )
                nc.vector.tensor_copy(dhT_sb[:, ko, :], ps_t)
                ps_v = prep_ps.tile([P, 1], F32, tag="ps_v")
                nc.tensor.matmul(
                    ps_v, lhsT=v_row[:, ko * P : (ko + 1) * P], rhs=one_sb,
                    start=True, stop=True,
                )
                nc.vector.tensor_copy(vT_sb[:, ko, :], ps_v)
            for dt in range(ko_d):
                ps_r = prep_ps.tile([P, b_loc], F32, tag="ps_r")
                for ko in range(ko_d):
                    nc.tensor.matmul(
                        ps_r,
                        lhsT=w_dec_sb[:, ko, dt * P : (dt + 1) * P],
                        rhs=dhT_sb[:, ko, :],
                        start=(ko == 0), stop=False,
                    )
                nc.tensor.matmul(
                    ps_r, lhsT=b_row[:, dt * P : (dt + 1) * P], rhs=ones_b,
                    start=False, stop=True,
                )
                nc.vector.tensor_copy(r_sb[:, dt, :], ps_r)

        # ---- main pools ----
        encT_pool = ctx.enter_context(tc.tile_pool(name="encT", bufs=2))
        energy_pool = ctx.enter_context(tc.tile_pool(name="energy", bufs=3))
        soft_pool = ctx.enter_context(tc.tile_pool(name="soft", bufs=2))
        ctxin_pool = ctx.enter_context(tc.tile_pool(name="ctxin", bufs=4))
        out_pool = ctx.enter_context(tc.tile_pool(name="outp", bufs=2))
        gemm_ps = ctx.enter_context(tc.tile_pool(name="gps", bufs=3, space="PSUM"))
        s_ps = ctx.enter_context(tc.tile_pool(name="sps", bufs=2, space="PSUM"))
        misc_ps = ctx.enter_context(tc.tile_pool(name="mps", bufs=1, space="PSUM"))
        ctx_ps = ctx.enter_context(tc.tile_pool(name="cps", bufs=1, space="PSUM"))

        for b in range(b_loc):
            # bf16 bounce of enc[b] in DRAM, then xbar-transposed tiles to SBUF
            enc_b16 = dram.tile([s, e], BF16, tag=f"encb16_{b}")
            nc.gpsimd.dma_start(out=enc_b16, in_=enc_p[b])
            encT = encT_pool.tile([P, ko_e, s], BF16, tag="encT")
            for eo in range(ko_e):
                nc.sync.dma_start(
                    out=encT[:, eo, :],
                    in_=enc_b16[:, eo * P : (eo + 1) * P],
                    transpose=True,
                )

            # additive mask bias: (m - 1) * 1e9  (0 -> -1e9, 1 -> 0)
            mi = soft_pool.tile([1, s], I32, tag="mi")
            nc.sync.dma_start(out=mi, in_=mask_p[b : b + 1, :])
            mb = soft_pool.tile([1, s], F32, tag="mb")
            nc.vector.tensor_copy(mb, mi)
            nc.vector.tensor_scalar(
                out=mb, in0=mb, scalar1=1e9, scalar2=1e9,
                op0=mybir.AluOpType.mult, op1=mybir.AluOpType.subtract,
            )

            E_b = soft_pool.tile([1, s], F32, tag="E_b")
            zs = soft_pool.tile([1, nch], F32, tag="zs")
            for n in range(nch):
                nsl = slice(n * ncs, (n + 1) * ncs)
                ps_s = s_ps.tile([1, ncs], F32, tag="ps_s")
                for dt in range(ko_d):
                    ps_g = gemm_ps.tile([P, ncs], F32, tag="ps_g")
                    for ko in range(ko_e):
                        nc.tensor.matmul(
                            ps_g,
                            lhsT=w_enc_sb[:, ko, dt * P : (dt + 1) * P],
                            rhs=encT[:, ko, nsl],
                            start=(ko == 0), stop=(ko == ko_e - 1),
                        )
                    energy = energy_pool.tile([P, ncs], BF16, tag="energy")
                    nc.scalar.activation(
                        energy, ps_g, Tanh, bias=r_sb[:, dt, b : b + 1], scale=1.0
                    )
                    nc.tensor.matmul(
                        ps_s, lhsT=vT_sb[:, dt, :], rhs=energy,
                        start=(dt == 0), stop=(dt == ko_d - 1),
                    )
                sp = soft_pool.tile([1, ncs], F32, tag="sp")
                nc.vector.tensor_add(sp, ps_s, mb[:, nsl])
                nc.scalar.activation(
                    E_b[:, nsl], sp, Exp, accum_out=zs[:, n : n + 1]
                )

            z_sb = soft_pool.tile([1, 1], F32, tag="z_sb")
            nc.vector.tensor_reduce(
                z_sb, zs, axis=mybir.AxisListType.X, op=mybir.AluOpType.add
            )
            rz = soft_pool.tile([1, 1], F32, tag="rz")
            nc.vector.reciprocal(rz, z_sb)

            attn_sb = out_pool.tile([1, s], F32, tag="attn_sb")
            nc.vector.tensor_scalar_mul(attn_sb, E_b, rz)
            nc.sync.dma_start(out=attn_p[b : b + 1, :], in_=attn_sb)

            # w^T tiles via fused transpose+scale matmul: (E chunk)^T @ (1/Z)
            wT_sb = out_pool.tile([P, st_n], BF16, tag="wT_sb")
            for g in range(0, st_n, 4):
                gn = min(4, st_n - g)
                ps_w = misc_ps.tile([P, 4], F32, tag="ps_w")
                for j in range(gn):
                    st = g + j
                    nc.tensor.matmul(
                        ps_w[:, j : j + 1],
                        lhsT=E_b[:, st * P : (st + 1) * P], rhs=rz,
                        start=True, stop=True,
                    )
                nc.vector.tensor_copy(wT_sb[:, g : g + gn], ps_w[:, 0:gn])

            # context[b] = sum_s w[s] * enc[b, s, :]
            ctx_sb = out_pool.tile([1, e], F32, tag="ctx_sb")
            ps_c = [
                ctx_ps.tile([1, ecs], F32, tag=f"ps_c{ec}", name=f"ps_c{ec}")
                for ec in range(ech)
            ]
            for st in range(st_n):
                nat = ctxin_pool.tile([P, e], BF16, tag="nat")
                nc.sync.dma_start(out=nat, in_=enc_b16[st * P : (st + 1) * P, :])
                for ec in range(ech):
                    nc.tensor.matmul(
                        ps_c[ec],
                        lhsT=wT_sb[:, st : st + 1],
                        rhs=nat[:, ec * ecs : (ec + 1) * ecs],
                        start=(st == 0), stop=(st == st_n - 1),
                    )
            for ec in range(ech):
                nc.vector.tensor_copy(ctx_sb[:, ec * ecs : (ec + 1) * ecs], ps_c[ec])
            nc.sync.dma_start(out=ctx_p[b : b + 1, :], in_=ctx_sb)

    return nc


_nc_cache = {}
LAST_RESULTS = None


def kernel(decoder_hidden, encoder_outputs, mask, W_attn, b_attn, v_w):
    global LAST_RESULTS
    _ensure_ntff_hook()
    from concourse.bass_utils import run_bass_kernel_spmd

    key = (B_LOC, S, E, D)
    if key not in _nc_cache:
        _nc_cache[key] = build_nc(*key)
    nc = _nc_cache[key]

    dh = np.ascontiguousarray(np.asarray(decoder_hidden, dtype=np.float32))
    enc = np.ascontiguousarray(np.asarray(encoder_outputs, dtype=np.float32))
    msk = np.ascontiguousarray(np.asarray(mask, dtype=np.int32))
    wat = np.ascontiguousarray(np.asarray(W_attn, dtype=np.float32))
    bat = np.ascontiguousarray(np.asarray(b_attn, dtype=np.float32)).reshape(1, D)
    vw = np.ascontiguousarray(np.asarray(v_w, dtype=np.float32)).reshape(1, D)

    in_maps = []
    for c in range(N_CORES):
        sl = slice(c * B_LOC, (c + 1) * B_LOC)
        in_maps.append(
            {
                "dh": dh[sl],
                "enc": enc[sl],
                "mask": msk[sl],
                "w_attn": wat,
                "b_attn": bat,
                "v_w": vw,
            }
        )

    res = run_bass_kernel_spmd(nc, in_maps, list(range(N_CORES)))
    LAST_RESULTS = res
    context = np.concatenate([res.results[c]["ctx_out"] for c in range(N_CORES)], 0)
    attn = np.concatenate([res.results[c]["attn_out"] for c in range(N_CORES)], 0)
    return context.astype(np.float32), attn.astype(np.float32)


# revision 13
# speedup vs baseline: 1.1785x; 1.1785x over previous
"""Bahdanau attention on 8 Trainium2 NeuronCores (Bass/Tile).

Data-parallel over batch: B=32 -> 4 batches per core; attn/v weights
replicated. Per core, per batch b:
  enc_proj^T[d, s] = W_enc^T @ enc[b]^T      (bf16 TensorE GEMM)
  energy = tanh(enc_proj^T + r_b)            (ScalarE, r_b = W_dec^T dh_b + b_attn)
  scores[s] = v^T @ energy                   (TensorE, M=1)
  E = exp(scores + (mask-1)*1e9), Z = sum E  (ScalarE exp + accum)
  attn = E / Z                               (VectorE)
  context = attn @ enc[b]                    (TensorE, M=1)

enc^T comes from an f32->bf16 cast DMA (SWDGE) into a DRAM bounce buffer
followed by hardware xbar transpose DMA (2-byte dtype only) back to SBUF.
"""

import os
import sys

import numpy as np

import concourse.bass as bass
import concourse.tile as tile
from concourse import mybir

F32 = mybir.dt.float32
BF16 = mybir.dt.bfloat16
I32 = mybir.dt.int32

N_CORES = 8
B, S, E, D = 32, 2048, 1024, 1024
B_LOC = B // N_CORES
P = 128


def _ensure_ntff_hook():
    """Register the axon NTFF profiling hook if the image lacks antenv.axon_hooks.

    Needed only when profiling (trace=True / BASS_TRACE=1); harmless otherwise.
    """
    try:
        import antenv.axon_hooks  # noqa: F401
        return
    except ImportError:
        pass
    try:
        import types

        import antenv
        from trn_agent_boot.trn_boot import _ntff_profile_via_ctypes

        mod = types.ModuleType("antenv.axon_hooks")
        _state = {"hook": None}

        def set_axon_ntff_profile_hook(hook):
            _state["hook"] = hook

        def get_axon_ntff_profile_hook():
            return _state["hook"]

        mod.set_axon_ntff_profile_hook = set_axon_ntff_profile_hook
        mod.get_axon_ntff_profile_hook = get_axon_ntff_profile_hook
        sys.modules["antenv.axon_hooks"] = mod
        antenv.axon_hooks = mod
        so_path = "/opt/axon/libaxon_pjrt.so"
        if os.path.exists(so_path):
            hook = _ntff_profile_via_ctypes(so_path)
            if hook is not None:
                set_axon_ntff_profile_hook(hook)
    except Exception:
        pass


def _split_multi_waits(nc):
    """Rewrite every instruction with >1 sem wait: keep one wait, hoist the
    rest onto same-engine NOPs inserted immediately before it.

    The pinned walrus codegen supports only ONE sync wait command per
    instruction ('Too many sync wait commands' otherwise); Tile's scheduler
    freely attaches several.
    """
    import bass_rust

    counter = 0
    for bassbb in nc.bb_map.values():
        bb = bassbb.bb
        out = []
        changed = False
        for inst in bb.instructions:
            si = inst.sync_info
            waits = list(si.on_wait) if si is not None else []
            if len(waits) > 1:
                changed = True
                for w in waits[:-1]:
                    counter += 1
                    n = mybir.InstNoOp(name=f"antws-{counter}")
                    n.engine = inst.engine
                    n.sync_info = bass_rust.SyncInfo(on_wait=[w], on_update=[])
                    nc.register_instruction(n, overwrite=True)
                    out.append(n)
                inst.sync_info = bass_rust.SyncInfo(
                    on_wait=[waits[-1]], on_update=list(si.on_update)
                )
            out.append(inst)
        if changed:
            bb.instructions = out


def _patch_tile_tail_drain():
    """Run _split_multi_waits after Tile's tail drain+barrier emission."""
    if getattr(tile.TileContext, "_ant_drain_patched", False):
        return
    orig = tile.TileContext._drain_and_barrier

    def _drain_and_barrier(self, tick_clock, wait_clock):
        orig(self, tick_clock, wait_clock)
        _split_multi_waits(self.nc)

    tile.TileContext._drain_and_barrier = _drain_and_barrier
    tile.TileContext._ant_drain_patched = True


def build_nc(b_loc=B_LOC, s=S, e=E, d=D):
    """Build the per-core Bass program (same SPMD program on all cores)."""
    _patch_tile_tail_drain()
    assert e % P == 0 and d % P == 0 and s % P == 0
    ko_e = e // P  # contraction tiles over enc dim
    ko_d = d // P  # tiles over proj dim d (and dec-hidden contraction)
    nch = max(1, s // 512)  # 512-row score chunks per batch
    ncs = s // nch
    st_n = s // P  # s-tiles per batch (context contraction)
    ech = max(1, e // 512)  # context output chunks
    ecs = e // ech

    nc = bass.Bass("TRN2")
    dh_p = nc.declare_dram_parameter("dh", [b_loc, d], F32, isOutput=False)
    enc_p = nc.declare_dram_parameter("enc", [b_loc, s, e], F32, isOutput=False)
    mask_p = nc.declare_dram_parameter("mask", [b_loc, s], I32, isOutput=False)
    wat_p = nc.declare_dram_parameter("w_attn", [d + e, d], F32, isOutput=False)
    bat_p = nc.declare_dram_parameter("b_attn", [1, d], F32, isOutput=False)
    vw_p = nc.declare_dram_parameter("v_w", [1, d], F32, isOutput=False)
    ctx_p = nc.declare_dram_parameter("ctx_out", [b_loc, e], F32, isOutput=True)
    attn_p = nc.declare_dram_parameter("attn_out", [b_loc, s], F32, isOutput=True)

    Tanh = mybir.ActivationFunctionType.Tanh
    Exp = mybir.ActivationFunctionType.Exp

    from contextlib import ExitStack

    with tile.TileContext(nc) as tc, ExitStack() as ctx:
        dram = ctx.enter_context(tc.tile_pool(name="dram", bufs=1, space="DRAM"))
        singles = ctx.enter_context(tc.tile_pool(name="singles", bufs=1))

        # enc bf16 bounce, grouped by 128-wide e-column block so both the
        # xbar-transpose source and the context natural reads are contiguous.
        # Batch 0 casts are issued first: batch 0's transposes gate the
        # first GEMM, and the SWDGE queue drains in issue order.
        enc_g = []
        for b in range(b_loc):
            g = dram.tile([ko_e, s, P], BF16, tag=f"encg{b}", name=f"encg{b}")
            enc_g.append(g)
            for eo in range(ko_e):
                nc.gpsimd.dma_start(
                    out=g[eo], in_=enc_p[b][:, eo * P : (eo + 1) * P]
                )

        w_enc_sb = singles.tile([P, ko_e, d], BF16)
        r_sb = singles.tile([P, ko_d, b_loc], F32)
        vT_sb = singles.tile([P, ko_d, 1], BF16)

        # ---- prep: r = W_dec^T dh^T + b_attn (as [d-part, b]), v^T tiles ----
        with (
            tc.tile_pool(name="prep", bufs=1) as prep,
            tc.tile_pool(name="prep_ps", bufs=2, space="PSUM") as prep_ps,
        ):
            # weights via HWDGE (f32) + DVE cast: keeps the SWDGE queue
            # free for the enc casts above
            w_enc_f = prep.tile([P, ko_e, d], F32)
            nc.sync.dma_start(
                out=w_enc_f,
                in_=wat_p[d : d + e, :].rearrange("(ko p) n -> p ko n", p=P),
            )
            for eo in range(ko_e):
                nc.vector.tensor_copy(w_enc_sb[:, eo, :], w_enc_f[:, eo, :])
            w_dec_f = prep.tile([P, ko_d, d], F32)
            nc.sync.dma_start(
                out=w_dec_f,
                in_=wat_p[0:d, :].rearrange("(ko p) n -> p ko n", p=P),
            )
            w_dec_sb = prep.tile([P, ko_d, d], BF16)
            for ko in range(ko_d):
                nc.vector.tensor_copy(w_dec_sb[:, ko, :], w_dec_f[:, ko, :])
            dh_sb = prep.tile([b_loc, d], F32)
            nc.sync.dma_start(out=dh_sb, in_=dh_p[:, :])
            eye_i = prep.tile([b_loc, b_loc], I32)
            nc.gpsimd.iota(eye_i, pattern=[[-1, b_loc]], base=0, channel_multiplier=1)
            eye_sb = prep.tile([b_loc, b_loc], F32)
            nc.vector.tensor_scalar(
                out=eye_sb, in0=eye_i, scalar1=0, scalar2=None,
                op0=mybir.AluOpType.is_equal,
            )
            one_sb = prep.tile([1, 1], F32)
            nc.vector.memset(one_sb, 1.0)
            ones_b = prep.tile([1, b_loc], BF16)
            nc.vector.memset(ones_b, 1.0)
            b_row_f = prep.tile([1, d], F32)
            nc.sync.dma_start(out=b_row_f, in_=bat_p[:, :])
            b_row = prep.tile([1, d], BF16)
            nc.vector.tensor_copy(b_row, b_row_f)
            v_row = prep.tile([1, d], F32)
            nc.sync.dma_start(out=v_row, in_=vw_p[:, :])

            dhT_sb = prep.tile([P, ko_d, b_loc], BF16)
            for ko in range(ko_d):
                ps_t = prep_ps.tile([P, b_loc], F32, tag="ps_t")
                nc.tensor.matmul(
                    ps_t, lhsT=dh_sb[:, ko * P : (ko + 1) * P], rhs=eye_sb,
                    start=True, stop=True,
                )
                nc.vector.tensor_copy(dhT_sb[:, ko, :], ps_t)
                ps_v = prep_ps.tile([P, 1], F32, tag="ps_v")
                nc.tensor.matmul(
                    ps_v, lhsT=v_row[:, ko * P : (ko + 1) * P], rhs=one_sb,
                    start=True, stop=True,
                )
                nc.vector.tensor_copy(vT_sb[:, ko, :], ps_v)
            for dt in range(ko_d):
                ps_r = prep_ps.tile([P, b_loc], F32, tag="ps_r")
                for ko in range(ko_d):
                    nc.tensor.matmul(
                        ps_r,
                        lhsT=w_dec_sb[:, ko, dt * P : (dt + 1) * P],
                        rhs=dhT_sb[:, ko, :],
                        start=(ko == 0), stop=False,
                    )
                nc.tensor.matmul(
                    ps_r, lhsT=b_row[:, dt * P : (dt + 1) * P], rhs=ones_b,
                    start=False, stop=True,
                )
                nc.vector.tensor_copy(r_sb[:, dt, :], ps_r)

        # ---- main pools ----
        encT_pool = ctx.enter_context(tc.tile_pool(name="encT", bufs=2))
        energy_pool = ctx.enter_context(tc.tile_pool(name="energy", bufs=3))
        soft_pool = ctx.enter_context(tc.tile_pool(name="soft", bufs=2))
        ctxin_pool = ctx.enter_context(tc.tile_pool(name="ctxin", bufs=1))
        out_pool = ctx.enter_context(tc.tile_pool(name="outp", bufs=2))
        gemm_ps = ctx.enter_context(tc.tile_pool(name="gps", bufs=3, space="PSUM"))
        s_ps = ctx.enter_context(tc.tile_pool(name="sps", bufs=2, space="PSUM"))
        misc_ps = ctx.enter_context(tc.tile_pool(name="mps", bufs=1, space="PSUM"))
        ctx_ps = ctx.enter_context(tc.tile_pool(name="cps", bufs=1, space="PSUM"))

        for b in range(b_loc):
            enc_b16 = enc_g[b]
            encT = encT_pool.tile([P, ko_e, s], BF16, tag="encT")
            for eo in range(ko_e):
                nc.sync.dma_start(
                    out=encT[:, eo, :], in_=enc_b16[eo], transpose=True
                )

            # additive mask bias: (m - 1) * 1e9  (0 -> -1e9, 1 -> 0)
            mi = soft_pool.tile([1, s], I32, tag="mi")
            nc.sync.dma_start(out=mi, in_=mask_p[b : b + 1, :])
            mb = soft_pool.tile([1, s], F32, tag="mb")
            nc.vector.tensor_copy(mb, mi)
            nc.vector.tensor_scalar(
                out=mb, in0=mb, scalar1=1e9, scalar2=1e9,
                op0=mybir.AluOpType.mult, op1=mybir.AluOpType.subtract,
            )

            E_b = soft_pool.tile([1, s], F32, tag="E_b")
            zs = soft_pool.tile([1, nch], F32, tag="zs")
            for n in range(nch):
                nsl = slice(n * ncs, (n + 1) * ncs)
                ps_s = s_ps.tile([1, ncs], F32, tag="ps_s")
                for dt in range(ko_d):
                    ps_g = gemm_ps.tile([P, ncs], F32, tag="ps_g")
                    for ko in range(ko_e):
                        nc.tensor.matmul(
                            ps_g,
                            lhsT=w_enc_sb[:, ko, dt * P : (dt + 1) * P],
                            rhs=encT[:, ko, nsl],
                            start=(ko == 0), stop=(ko == ko_e - 1),
                        )
                    energy = energy_pool.tile([P, ncs], BF16, tag="energy")
                    nc.scalar.activation(
                        energy, ps_g, Tanh, bias=r_sb[:, dt, b : b + 1], scale=1.0
                    )
                    nc.tensor.matmul(
                        ps_s, lhsT=vT_sb[:, dt, :], rhs=energy,
                        start=(dt == 0), stop=(dt == ko_d - 1),
                    )
                sp = soft_pool.tile([1, ncs], F32, tag="sp")
                nc.vector.tensor_add(sp, ps_s, mb[:, nsl])
                nc.scalar.activation(
                    E_b[:, nsl], sp, Exp, accum_out=zs[:, n : n + 1]
                )

            z_sb = soft_pool.tile([1, 1], F32, tag="z_sb")
            nc.vector.tensor_reduce(
                z_sb, zs, axis=mybir.AxisListType.X, op=mybir.AluOpType.add
            )
            rz = soft_pool.tile([1, 1], F32, tag="rz")
            nc.vector.reciprocal(rz, z_sb)

            attn_sb = out_pool.tile([1, s], F32, tag="attn_sb")
            nc.vector.tensor_scalar_mul(attn_sb, E_b, rz)
            nc.sync.dma_start(out=attn_p[b : b + 1, :], in_=attn_sb)

            # w^T tiles via fused transpose+scale matmul: (E chunk)^T @ (1/Z)
            wT_sb = out_pool.tile([P, st_n], BF16, tag="wT_sb")
            for g in range(0, st_n, 4):
                gn = min(4, st_n - g)
                ps_w = misc_ps.tile([P, 4], F32, tag="ps_w")
                for j in range(gn):
                    st = g + j
                    nc.tensor.matmul(
                        ps_w[:, j : j + 1],
                        lhsT=E_b[:, st * P : (st + 1) * P], rhs=rz,
                        start=True, stop=True,
                    )
                nc.vector.tensor_copy(wT_sb[:, g : g + gn], ps_w[:, 0:gn])

            # context[b] = sum_s w[s] * enc[b, s, :]; natural-layout enc
            # re-read from the grouped bounce, one contiguous DMA per e-block
            nat = ctxin_pool.tile([P, st_n, ko_e, P], BF16, tag="nat")
            for eo in range(ko_e):
                nc.sync.dma_start(
                    out=nat[:, :, eo, :],
                    in_=enc_b16[eo].rearrange("(st p) e -> p st e", p=P),
                )
            epb = ecs // P  # e-blocks per output chunk
            ctx_sb = out_pool.tile([1, e], F32, tag="ctx_sb")
            ps_c = [
                ctx_ps.tile([1, ecs], F32, tag=f"ps_c{ec}", name=f"ps_c{ec}")
                for ec in range(ech)
            ]
            for st in range(st_n):
                for ec in range(ech):
                    nc.tensor.matmul(
                        ps_c[ec],
                        lhsT=wT_sb[:, st : st + 1],
                        rhs=nat[:, st, ec * epb : (ec + 1) * epb, :],
                        start=(st == 0), stop=(st == st_n - 1),
                    )
            for ec in range(ech):
                nc.vector.tensor_copy(ctx_sb[:, ec * ecs : (ec + 1) * ecs], ps_c[ec])
            nc.sync.dma_start(out=ctx_p[b : b + 1, :], in_=ctx_sb)

    return nc


_nc_cache = {}
LAST_RESULTS = None


def kernel(decoder_hidden, encoder_outputs, mask, W_attn, b_attn, v_w):
    global LAST_RESULTS
    _ensure_ntff_hook()
    from concourse.bass_utils import run_bass_kernel_spmd

    key = (B_LOC, S, E, D)
    if key not in _nc_cache:
        _nc_cache[key] = build_nc(*key)
    nc = _nc_cache[key]

    dh = np.ascontiguousarray(np.asarray(decoder_hidden, dtype=np.float32))
    enc = np.ascontiguousarray(np.asarray(encoder_outputs, dtype=np.float32))
    msk = np.ascontiguousarray(np.asarray(mask, dtype=np.int32))
    wat = np.ascontiguousarray(np.asarray(W_attn, dtype=np.float32))
    bat = np.ascontiguousarray(np.asarray(b_attn, dtype=np.float32)).reshape(1, D)
    vw = np.ascontiguousarray(np.asarray(v_w, dtype=np.float32)).reshape(1, D)

    in_maps = []
    for c in range(N_CORES):
        sl = slice(c * B_LOC, (c + 1) * B_LOC)
        in_maps.append(
            {
                "dh": dh[sl],
                "enc": enc[sl],
                "mask": msk[sl],
                "w_attn": wat,
                "b_attn": bat,
                "v_w": vw,
            }
        )

    res = run_bass_kernel_spmd(nc, in_maps, list(range(N_CORES)))
    LAST_RESULTS = res
    context = np.concatenate([res.results[c]["ctx_out"] for c in range(N_CORES)], 0)
    attn = np.concatenate([res.results[c]["attn_out"] for c in range(N_CORES)], 0)
    return context.astype(np.float32), attn.astype(np.float32)
